# revision 1
# baseline (speedup 1.0000x reference)
"""Trainium2 Bass kernel for MockBitNetLayer:

    scale = mean(|W|, axis=1)            # [O, 1]
    y = x @ (sign(W) * scale).T + bias   # [T, O]

Strategy (column-parallel over 8 NeuronCores), v4:
  - Each core owns an O/8 = 2048-column shard of W.T and bias; x is
    shared.  Host-side input marshaling (transpose + dtype cast + tile
    layout) is done in numpy during sharding; all model arithmetic
    (sign, |W| mean, matmul, scale/bias) runs on device.
  - x arrives as x.T in two pieces: the first NK8*128 contraction rows
    cast to fp8e4, the rest to fp16.  W arrives both as W.T (k-major,
    for the sign path: no on-device transpose needed) and as W
    (o-major, for the per-row |W| mean on the vector engine).
  - sign(W).T tiles are produced by the scalar engine straight from
    k-major W.T slabs: fp8e4 for the fp8 k-range, fp16 for the rest.
    The PE runs nothing but the matmul stream.
  - Matmul stream per 512-token chunk: fp16 MMs for the fp16 k-tiles
    plus fp8 DoubleRow MMs (2 k-tiles per MM, measured at the same
    216 ns as a plain MM => 2x throughput) accumulating into PSUM.
  - Eviction fuses scale/bias on the scalar engine: yt = psum*scale+b,
    written as y.T shards; host concatenates + transposes.
  - Error budget: fp8e4 x-quantization on NK8/32 of the contraction:
    measured 1.87e-2 at NK8=16 against the fp32 reference (tolerance
    2e-2); the fp16 part contributes ~2e-4.
"""

import os
import sys

for _p in ("/opt/trn_rl_repo", "/root/.axon_site/_ro/trn_rl_repo"):
    if os.path.isdir(_p) and _p not in sys.path:
        sys.path.insert(0, _p)

import numpy as np
import ml_dtypes

import concourse.bacc as bacc
import concourse.mybir as mybir
import concourse.tile as tile
from concourse.bass import ds
from concourse.bass_utils import run_bass_kernel_spmd

P = 128
N_CORES = 8

T_FULL = 8192
K_FULL = 4096
O_FULL = 16384

NK8 = int(os.environ.get("NK8T", "16"))  # fp8 k-tiles (even, 0..32)
TCH = 512


def build_kernel_body(tc, xt8, xt16, wt, w, b, yt, T, K, O, nk8):
    nc = tc.nc
    f32 = mybir.dt.float32
    f16 = mybir.dt.float16
    f8 = mybir.dt.float8e4
    bf16 = mybir.dt.bfloat16

    KT = K // P            # 32 k tiles
    KT16 = KT - nk8        # fp16 k tiles
    NPAIR = nk8 // 2       # fp8 DoubleRow pairs
    OT = O // P            # 16 o tiles
    OS = O // 512          # 512-wide o slabs
    NTCH = T // TCH        # token chunks
    KS = K // 512          # 512-wide k slabs per W row tile
    OB = 4                 # o tiles per psum block
    NOB = OT // OB

    with (
        tc.tile_pool(name="const", bufs=1) as const_pool,
        tc.tile_pool(name="wstage", bufs=6) as wstage,
        tc.tile_pool(name="astage", bufs=3) as astage,
        tc.tile_pool(name="swt", bufs=1) as swt_pool,
        tc.tile_pool(name="xt", bufs=3) as xt_pool,
        tc.tile_pool(name="out", bufs=4) as out_pool,
        tc.tile_pool(name="psum_mm", bufs=8, space="PSUM") as psum_mm,
    ):
        scale_sb = const_pool.tile([P, OT], f32)
        bias_sb = const_pool.tile([P, OT], f32)
        partials = const_pool.tile([P, KS], f32)

        swt8 = swt_pool.tile([P, max(nk8, 1), O], f8)
        swt16 = swt_pool.tile([P, max(KT16, 1), O], f16)

        # ---- chunk-0 x tiles first: their DMAs must not queue behind W ----
        # x arrives chunk-major ([NTCH, P, kt, TCH]) so a chunk is one
        # big contiguous DMA.
        def load_x(c):
            tiles = [None, None]
            if nk8:
                tiles[0] = xt_pool.tile([P, nk8, TCH], f8, tag="x8", name=f"x8_{c}")
                nc.sync.dma_start(tiles[0], xt8[c])
            if KT16:
                tiles[1] = xt_pool.tile([P, KT16, TCH], f16, tag="x16", name=f"x16_{c}")
                nc.scalar.dma_start(tiles[1], xt16[c])
            return tiles

        x_pre = {c: load_x(c) for c in range(min(2, NTCH))}

        # ---- sign path: k-major bf16 W.T slabs -> sign -> resident tiles --
        # two o-halves so the first MM blocks' weights are ready first;
        # fp16 k-tiles first within a half (MMs consume them first).
        OH = O // 2
        kt_order = list(range(nk8, KT)) + list(range(nk8))
        for oh in range(2):
            for i, kt in enumerate(kt_order):
                ws = wstage.tile([P, OH], bf16, tag="ws")
                q = nc.sync if (i % 2 == 0) else nc.scalar
                q.dma_start(ws, wt[ds(kt * P, P), ds(oh * OH, OH)])
                if kt < nk8:
                    nc.scalar.sign(swt8[:, kt, ds(oh * OH, OH)], ws)
                else:
                    nc.scalar.sign(swt16[:, kt - nk8, ds(oh * OH, OH)], ws)

        # ---- scale path: o-major bf16 W rows -> |.| row sums (DVE) ----
        for ot in range(OT):
            nc.gpsimd.dma_start(
                bias_sb[:, ot : ot + 1],
                b[ds(ot * P, P)].rearrange("(p one) -> p one", one=1),
            )
            for kh in range(2):
                wa = astage.tile([P, K // 2], bf16, tag="wa")
                nc.gpsimd.dma_start(wa, w[ds(ot * P, P), ds(kh * K // 2, K // 2)])
                nc.vector.tensor_reduce(
                    out=partials[:, kh : kh + 1],
                    in_=wa,
                    axis=mybir.AxisListType.X,
                    op=mybir.AluOpType.add,
                    apply_absolute_value=True,
                )
            stot = const_pool.tile([P, 1], f32, tag="stot")
            nc.vector.tensor_reduce(
                out=stot, in_=partials[:, 0:2], axis=mybir.AxisListType.X,
                op=mybir.AluOpType.add,
            )
            nc.scalar.mul(scale_sb[:, ot : ot + 1], stot, 1.0 / K)

        # ---- main loop over token chunks ----
        for c in range(NTCH):
            x8, x16 = x_pre.pop(c)
            if c + 2 < NTCH:
                x_pre[c + 2] = load_x(c + 2)
            for ob in range(NOB):
                psums = [
                    psum_mm.tile([P, TCH], f32, tag="acc", name=f"acc{oi}")
                    for oi in range(OB)
                ]
                n_mm = KT16 + NPAIR
                mm_i = 0
                for kt in range(KT16):
                    for oi in range(OB):
                        ot = ob * OB + oi
                        nc.tensor.matmul(
                            psums[oi],
                            lhsT=swt16[:, kt, ds(ot * P, P)],
                            rhs=x16[:, kt, :],
                            start=(mm_i == 0),
                            stop=(mm_i == n_mm - 1),
                        )
                    mm_i += 1
                for pr in range(NPAIR):
                    for oi in range(OB):
                        ot = ob * OB + oi
                        nc.tensor.matmul(
                            psums[oi],
                            lhsT=swt8[:, ds(2 * pr, 2), ds(ot * P, P)],
                            rhs=x8[:, ds(2 * pr, 2), :],
                            start=(mm_i == 0),
                            stop=(mm_i == n_mm - 1),
                            perf_mode=mybir.MatmulPerfMode.DoubleRow,
                        )
                    mm_i += 1
                for oi in range(OB):
                    ot = ob * OB + oi
                    out_sb = out_pool.tile([P, TCH], f32, name="out_sb")
                    nc.scalar.activation(
                        out_sb,
                        psums[oi],
                        mybir.ActivationFunctionType.Identity,
                        bias=bias_sb[:, ot : ot + 1],
                        scale=scale_sb[:, ot : ot + 1],
                    )
                    nc.gpsimd.dma_start(
                        yt[ds(ot * P, P), ds(c * TCH, TCH)], out_sb
                    )


def build_bass(T=T_FULL, K=K_FULL, O=O_FULL // N_CORES, nk8=NK8):
    nc = bacc.Bacc(trn_type="TRN2")
    f32 = mybir.dt.float32
    f16 = mybir.dt.float16
    f8 = mybir.dt.float8e4
    KT16 = K // P - nk8
    NTCH = T // TCH
    xt8 = (
        nc.dram_tensor("xt8", [NTCH, P, nk8, TCH], f8, kind="ExternalInput").ap()
        if nk8
        else None
    )
    xt16 = (
        nc.dram_tensor("xt16", [NTCH, P, KT16, TCH], f16, kind="ExternalInput").ap()
        if KT16
        else None
    )
    bf16 = mybir.dt.bfloat16
    wt = nc.dram_tensor("wt", [K, O], bf16, kind="ExternalInput").ap()
    w = nc.dram_tensor("w", [O, K], bf16, kind="ExternalInput").ap()
    b = nc.dram_tensor("b", [O], f32, kind="ExternalInput").ap()
    yt = nc.dram_tensor("yt", [O, T], f32, kind="ExternalOutput").ap()
    with tile.TileContext(nc) as tc:
        build_kernel_body(tc, xt8, xt16, wt, w, b, yt, T, K, O, nk8)
    nc.finalize()
    return nc


_CACHED_NC = None


def _get_nc():
    global _CACHED_NC
    if _CACHED_NC is None:
        _CACHED_NC = build_bass()
    return _CACHED_NC


def make_in_maps(x, weight, bias):
    x = np.asarray(x, dtype=np.float32)
    weight = np.ascontiguousarray(np.asarray(weight, dtype=np.float32))
    bias = np.ascontiguousarray(np.asarray(bias, dtype=np.float32))
    O = weight.shape[0] // N_CORES
    K = x.shape[1]
    T = x.shape[0]
    KT16 = K // P - NK8
    # host-side marshaling: transpose + cast + tile layout [128, kt, T]
    xt = np.ascontiguousarray(x.T)  # [K, T]
    base = {}
    NTCH = T // TCH
    if NK8:
        base["xt8"] = np.ascontiguousarray(
            xt[: NK8 * P].reshape(NK8, P, NTCH, TCH).transpose(2, 1, 0, 3)
        ).astype(ml_dtypes.float8_e4m3fn)
    if KT16:
        base["xt16"] = np.ascontiguousarray(
            xt[NK8 * P :].reshape(KT16, P, NTCH, TCH).transpose(2, 1, 0, 3)
        ).astype(np.float16)
    wtf = np.ascontiguousarray(weight.T).astype(ml_dtypes.bfloat16)  # [K, O_FULL]
    w16 = weight.astype(ml_dtypes.bfloat16)
    return [
        {
            **base,
            "wt": np.ascontiguousarray(wtf[:, c * O : (c + 1) * O]),
            "w": w16[c * O : (c + 1) * O],
            "b": bias[c * O : (c + 1) * O],
        }
        for c in range(N_CORES)
    ]


def kernel(x, weight, bias):
    nc = _get_nc()
    in_maps = make_in_maps(x, weight, bias)
    res = run_bass_kernel_spmd(nc, in_maps, list(range(N_CORES)))
    yt = np.concatenate([r["yt"] for r in res.results], axis=0)  # [O_FULL, T]
    return np.ascontiguousarray(yt.T)



# revision 3
# speedup vs baseline: 1.0859x; 1.0859x over previous
"""Trainium2 Bass kernel for MockBitNetLayer:

    scale = mean(|W|, axis=1)            # [O, 1]
    y = x @ (sign(W) * scale).T + bias   # [T, O]

Strategy (column-parallel over 8 NeuronCores), v5:
  - Each core owns an O/8 = 2048-column shard of W.T and bias; x is
    shared.  Host-side input marshaling (transpose + dtype cast + tile
    layout) is done in numpy during sharding; all model arithmetic
    (sign, |W| mean, matmul, scale/bias) runs on device.
  - Precision split over the contraction: the first NK8*128 rows of x
    in fp8e4 (consumed by DoubleRow MMs, 2 k-tiles per 216 ns slot),
    the rest in fp16 (1 k-tile per slot).  NK8=18 measures 1.99e-2
    against the fp32 reference (tolerance 2e-2); predicted exactly by
    a host-side numpy simulation of the quantization chain.
  - W arrives twice: k-major fp8e5 (sign path; values that would round
    to zero/denormal are host-fixed to +-2^-14 so sign() is exact) and
    o-major fp8e4 (scale path; mean|W| error ~0.06%, negligible).
  - Engine/queue split kills the v4 startup starvation (first MM was
    at 37 us, ~105 us of ramp losses):
      sync    : 64 W.T slab DMAs (nothing else -> no head-of-line)
      scalar  : 64 sign ops only
      vector  : |W| row reduces + 1/K -> then all psum evictions
                (tensor_scalar psum*scale+bias -> f16) + y DMAs
      gpsimd  : x chunk DMAs, bias, o-major W DMAs
      tensor  : nothing but the MM stream
  - Chunk 0 runs (ob0,ob1)/(ob2,ob3) interleaved across 8 PSUM banks,
    halving the sign-supply rate the ACT engine must sustain while the
    slab pipeline warms; later chunks use the rolling 4-bank pattern.
  - y is written as f16 (halves output traffic; +3e-4 error).
"""

import os
import sys

for _p in ("/opt/trn_rl_repo", "/root/.axon_site/_ro/trn_rl_repo"):
    if os.path.isdir(_p) and _p not in sys.path:
        sys.path.insert(0, _p)

import numpy as np
import ml_dtypes

import concourse.bacc as bacc
import concourse.mybir as mybir
import concourse.tile as tile
from concourse.bass import ds
from concourse.bass_utils import run_bass_kernel_spmd

P = 128
N_CORES = 8

T_FULL = 8192
K_FULL = 4096
O_FULL = 16384

NK8 = int(os.environ.get("NK8T", "18"))  # fp8 k-tiles (even, 0..32)
TCH = 512


def build_kernel_body(tc, xt8, xt16, wt, w8, b, yt, T, K, O, nk8):
    nc = tc.nc
    f32 = mybir.dt.float32
    f16 = mybir.dt.float16
    f8 = mybir.dt.float8e4
    f8w = mybir.dt.float8e5

    KT = K // P            # 32 k tiles
    KT16 = KT - nk8        # fp16 k tiles
    NPAIR = nk8 // 2       # fp8 DoubleRow pairs
    OT = O // P            # 16 o tiles
    NTCH = T // TCH        # token chunks
    OB = 4                 # o tiles per psum block (steady state)
    NOB = OT // OB
    XH = (KT16 + 1) // 2 if KT16 else 0  # x16 part-a size (k tiles)

    mult = mybir.AluOpType.mult
    addop = mybir.AluOpType.add

    with (
        tc.tile_pool(name="const", bufs=1) as const_pool,
        tc.tile_pool(name="wstage", bufs=6) as wstage,
        tc.tile_pool(name="astage", bufs=2) as astage,
        tc.tile_pool(name="swt", bufs=1) as swt_pool,
        tc.tile_pool(name="xt", bufs=3) as xt_pool,
        tc.tile_pool(name="out", bufs=8) as out_pool,
        tc.tile_pool(name="psum_mm", bufs=8, space="PSUM") as psum_mm,
    ):
        scale_sb = const_pool.tile([P, OT], f32)
        bias_sb = const_pool.tile([P, OT], f32)

        swt8 = swt_pool.tile([P, max(nk8, 1), O], f8)
        swt16 = swt_pool.tile([P, max(KT16, 1), O], f16)

        # ---- x chunks: gpsimd queue; x16 split so the first MMs' rows
        # land before the whole chunk does ----
        def load_x(c):
            t16a = t16b = t8 = None
            if KT16:
                t16a = xt_pool.tile([P, XH, TCH], f16, tag="x16a", name=f"x16a_{c}")
                nc.gpsimd.dma_start(t16a, xt16[c][:, ds(0, XH), :])
                if KT16 > XH:
                    t16b = xt_pool.tile(
                        [P, KT16 - XH, TCH], f16, tag="x16b", name=f"x16b_{c}"
                    )
                    nc.gpsimd.dma_start(t16b, xt16[c][:, ds(XH, KT16 - XH), :])
            if nk8:
                t8 = xt_pool.tile([P, nk8, TCH], f8, tag="x8", name=f"x8_{c}")
                nc.gpsimd.dma_start(t8, xt8[c])
            return (t16a, t16b, t8)

        def rhs16(xp, kt):
            if kt < XH:
                return xp[0][:, kt, :]
            return xp[1][:, kt - XH, :]

        x_pre = {0: load_x(0)}

        # ---- sign path: k-major fp8e5 slabs -> ACT sign -> resident ----
        # o-halves; fp16 k-tiles first within a half (MM consumption order).
        OH = O // 2
        kt_order = list(range(nk8, KT)) + list(range(nk8))
        for oh in range(2):
            for kt in kt_order:
                ws = wstage.tile([P, OH], f8w, tag="ws")
                nc.sync.dma_start(ws, wt[ds(kt * P, P), ds(oh * OH, OH)])
                if kt < nk8:
                    nc.scalar.sign(swt8[:, kt, ds(oh * OH, OH)], ws)
                else:
                    nc.scalar.sign(swt16[:, kt - nk8, ds(oh * OH, OH)], ws)

        # ---- scale path: o-major fp8e4 rows -> DVE |.| row mean ----
        for ot in range(OT):
            nc.gpsimd.dma_start(
                bias_sb[:, ds(ot, 1)],
                b[ds(ot * P, P)].rearrange("(p one) -> p one", one=1),
            )
            wa = astage.tile([P, K], f8, tag="wa")
            nc.gpsimd.dma_start(wa, w8[ds(ot * P, P), :])
            stot = const_pool.tile([P, 1], f32, tag="stot")
            nc.vector.tensor_reduce(
                out=stot,
                in_=wa,
                axis=mybir.AxisListType.X,
                op=addop,
                apply_absolute_value=True,
            )
            nc.vector.tensor_scalar_mul(scale_sb[:, ds(ot, 1)], stot, 1.0 / K)

        x_pre[1] = load_x(1)

        def evict(psum, ot, c):
            out_sb = out_pool.tile([P, TCH], f16, name="osb")
            nc.vector.tensor_scalar(
                out_sb,
                psum,
                scale_sb[:, ds(ot, 1)],
                bias_sb[:, ds(ot, 1)],
                mult,
                addop,
            )
            nc.scalar.dma_start(yt[ds(ot * P, P), ds(c * TCH, TCH)], out_sb)

        def mm_group(psums, ots, xp):
            n_units = KT16 + NPAIR
            u = 0
            for kt in range(KT16):
                for psum, ot in zip(psums, ots):
                    nc.tensor.matmul(
                        psum,
                        lhsT=swt16[:, kt, ds(ot * P, P)],
                        rhs=rhs16(xp, kt),
                        start=(u == 0),
                        stop=(u == n_units - 1),
                    )
                u += 1
            for pr in range(NPAIR):
                for psum, ot in zip(psums, ots):
                    nc.tensor.matmul(
                        psum,
                        lhsT=swt8[:, ds(2 * pr, 2), ds(ot * P, P)],
                        rhs=xp[2][:, ds(2 * pr, 2), :],
                        start=(u == 0),
                        stop=(u == n_units - 1),
                        perf_mode=mybir.MatmulPerfMode.DoubleRow,
                    )
                u += 1

        # ---- main loop over token chunks ----
        for c in range(NTCH):
            xp = x_pre.pop(c)
            if c + 2 < NTCH:
                x_pre[c + 2] = load_x(c + 2)
            if c == 0:
                # ob-pairs across all 8 psum banks: halves the rate at
                # which fresh sign slabs are consumed during warmup.
                for obp in range(NOB // 2):
                    ots = list(range(obp * 2 * OB, (obp + 1) * 2 * OB))
                    psums = [
                        psum_mm.tile([P, TCH], f32, tag="acc", name=f"acc{i}")
                        for i in range(2 * OB)
                    ]
                    mm_group(psums, ots, xp)
                    for psum, ot in zip(psums, ots):
                        evict(psum, ot, c)
            else:
                for ob in range(NOB):
                    ots = list(range(ob * OB, (ob + 1) * OB))
                    psums = [
                        psum_mm.tile([P, TCH], f32, tag="acc", name=f"acc{i}")
                        for i in range(OB)
                    ]
                    mm_group(psums, ots, xp)
                    for psum, ot in zip(psums, ots):
                        evict(psum, ot, c)


def build_bass(T=T_FULL, K=K_FULL, O=O_FULL // N_CORES, nk8=NK8):
    nc = bacc.Bacc(trn_type="TRN2")
    f32 = mybir.dt.float32
    f16 = mybir.dt.float16
    f8 = mybir.dt.float8e4
    f8w = mybir.dt.float8e5
    KT16 = K // P - nk8
    NTCH = T // TCH
    xt8 = (
        nc.dram_tensor("xt8", [NTCH, P, nk8, TCH], f8, kind="ExternalInput").ap()
        if nk8
        else None
    )
    xt16 = (
        nc.dram_tensor("xt16", [NTCH, P, KT16, TCH], f16, kind="ExternalInput").ap()
        if KT16
        else None
    )
    wt = nc.dram_tensor("wt", [K, O], f8w, kind="ExternalInput").ap()
    w8 = nc.dram_tensor("w8", [O, K], f8, kind="ExternalInput").ap()
    b = nc.dram_tensor("b", [O], f32, kind="ExternalInput").ap()
    yt = nc.dram_tensor("yt", [O, T], f16, kind="ExternalOutput").ap()
    with tile.TileContext(nc) as tc:
        build_kernel_body(tc, xt8, xt16, wt, w8, b, yt, T, K, O, nk8)
    nc.finalize()
    return nc


_CACHED_NC = None


def _get_nc():
    global _CACHED_NC
    if _CACHED_NC is None:
        _CACHED_NC = build_bass()
    return _CACHED_NC


def make_in_maps(x, weight, bias):
    x = np.asarray(x, dtype=np.float32)
    weight = np.ascontiguousarray(np.asarray(weight, dtype=np.float32))
    bias = np.ascontiguousarray(np.asarray(bias, dtype=np.float32))
    O = weight.shape[0] // N_CORES
    K = x.shape[1]
    T = x.shape[0]
    KT16 = K // P - NK8
    # host-side marshaling: transpose + cast + tile layout
    xt = np.ascontiguousarray(x.T)  # [K, T]
    base = {}
    NTCH = T // TCH
    if NK8:
        base["xt8"] = np.ascontiguousarray(
            xt[: NK8 * P].reshape(NK8, P, NTCH, TCH).transpose(2, 1, 0, 3)
        ).astype(ml_dtypes.float8_e4m3fn)
    if KT16:
        base["xt16"] = np.ascontiguousarray(
            xt[NK8 * P :].reshape(KT16, P, NTCH, TCH).transpose(2, 1, 0, 3)
        ).astype(np.float16)
    # sign path: k-major fp8e5; values that would land in the denormal/
    # zero range are pinned to +-2^-14 (min normal) so sign() on device
    # is exactly sign(W).
    wtf = np.ascontiguousarray(weight.T)  # [K, O_FULL] f32
    wt8 = wtf.astype(ml_dtypes.float8_e5m2)
    tiny = np.abs(wtf) < np.float32(2.0**-14)
    if tiny.any():
        fix = np.copysign(np.float32(2.0**-14), wtf).astype(ml_dtypes.float8_e5m2)
        wt8 = np.where(tiny, fix, wt8)
    # scale path: o-major fp8e4
    w8 = weight.astype(ml_dtypes.float8_e4m3fn)  # [O_FULL, K]
    return [
        {
            **base,
            "wt": np.ascontiguousarray(wt8[:, c * O : (c + 1) * O]),
            "w8": np.ascontiguousarray(w8[c * O : (c + 1) * O]),
            "b": bias[c * O : (c + 1) * O],
        }
        for c in range(N_CORES)
    ]


def kernel(x, weight, bias):
    nc = _get_nc()
    in_maps = make_in_maps(x, weight, bias)
    res = run_bass_kernel_spmd(nc, in_maps, list(range(N_CORES)))
    yt = np.concatenate([r["yt"] for r in res.results], axis=0)  # [O_FULL, T] f16
    return np.ascontiguousarray(yt.T.astype(np.float32))
